# revision 3
# baseline (speedup 1.0000x reference)
"""Trainium2 Bass kernel for nn_ContrastiveLoss_70712341561919.

Strategy (data-parallel over the B=32 video axis, 4 videos per core):

The heavy tensor is video_feats (32, 256, 64, 64) = 128 MiB.  All four loss
terms only need, per video b, the matrix of exp-scores

    E[b, r, n] = exp( s_r . vhat_b[:, n] / 0.1 )

where vhat_b[:, n] = video_feats[b, :, n] / ||video_feats[b, :, n]|| and the
rows s_r are the 32 normalized query vectors followed by the (few) normalized
top-k "tv" vectors of the sentences belonging to video b.  Each core:

  * loads its 4 videos' (256, 4096) matrices (16 MiB),
  * computes per-column sum-of-squares with an elementwise square plus a
    ones-vector matmul (contraction over the channel/partition axis),
  * rnorm10 = 10 / sqrt(ss)  (sqrt with scale=0.01 then reciprocal) - packed
    (128 cols, 32 blocks) per video so these are single instructions,
  * per 128-column block: matmul  A_block^T @ [qf | tv]  -> (128, ROWS) scores
    in PSUM, per-partition scale by rnorm10 (column norm lives on the
    partition axis in this orientation), exp on the scalar engine,
  * DMAs the exp-score matrix back out (f32).

The host then performs only cheap masked reductions over exp-scores plus the
tiny (T*K)-sized log/mean terms, replicating the reference exactly.  No
cross-core communication is needed: every core receives the full (tiny)
query/sentence tensors, and the cross-batch sums are reductions over the
per-video exp-score partials which the host combines.
"""

import os
import sys

import numpy as np

for _p in ("/root/.axon_site/_ro/trn_rl_repo", "/opt/trn_rl_repo"):
    if os.path.isdir(_p) and _p not in sys.path:
        sys.path.append(_p)

import concourse.bacc as bacc
import concourse.tile as tile
from concourse import mybir
from concourse.bass_utils import run_bass_kernel_spmd

# Problem constants (hardcoded per spec)
B = 32
C = 256
D = 64
N = D * D  # 4096
K = 3
TT = 0.1  # T_V == T_Q == 0.1
NEG_IOU = 0.5
NCORES = 8
VPC = B // NCORES  # videos per core

LAST_EXEC_NS = None
LAST_RESULTS = None

_PROGRAM_CACHE = {}


def _build_program(rows):
    """Build the SPMD Bass/Tile program (identical on all 8 cores).

    Inputs  : vf  (VPC, 256, 4096) f32   - this core's raw video features
              lhs (VPC, 2, 128, rows) f32 - [qf | tv] row vectors, chunked on C
    Outputs : eout (VPC, NGRP, 128, BPG*rows) f32 - exp(score/T) per column
    """
    f32 = mybir.dt.float32
    nblk = N // 128  # 32 column blocks per video
    # blocks per PSUM group: keep a group within one 2KB PSUM bank
    bpg = 1
    for cand in (32, 16, 8, 4, 2, 1):
        if cand * rows * 4 <= 2048 and nblk % cand == 0:
            bpg = cand
            break
    ngrp = nblk // bpg

    nc = bacc.Bacc("TRN2", target_bir_lowering=False, debug=False)
    vf_in = nc.dram_tensor("vf", [VPC, C, N], f32, kind="ExternalInput")
    lhs_in = nc.dram_tensor("lhs", [VPC, 2, 128, rows], f32, kind="ExternalInput")
    eout = nc.dram_tensor(
        "eout", [VPC, ngrp, 128, bpg * rows], f32, kind="ExternalOutput"
    )

    with tile.TileContext(nc) as tc:
        with (
            tc.tile_pool(name="a", bufs=4) as a_pool,
            tc.tile_pool(name="sq", bufs=2) as sq_pool,
            tc.tile_pool(name="lhsp", bufs=2) as lhs_pool,
            tc.tile_pool(name="rn", bufs=2) as rn_pool,
            tc.tile_pool(name="rn2", bufs=2) as rn2_pool,
            tc.tile_pool(name="sc", bufs=4) as sc_pool,
            tc.tile_pool(name="e", bufs=4) as e_pool,
            tc.tile_pool(name="consts", bufs=1) as const_pool,
            tc.tile_pool(name="pss", bufs=2, space="PSUM") as pss_pool,
            tc.tile_pool(name="ps", bufs=4, space="PSUM") as ps_pool,
        ):
            ones_t = const_pool.tile([128, 1], f32)
            nc.vector.memset(ones_t, 1.0)

            for v in range(VPC):
                a0 = a_pool.tile([128, N], f32)
                nc.sync.dma_start(out=a0, in_=vf_in[v, 0:128, :])
                a1 = a_pool.tile([128, N], f32)
                nc.sync.dma_start(out=a1, in_=vf_in[v, 128:256, :])
                lh = lhs_pool.tile([128, 2, rows], f32)
                nc.scalar.dma_start(out=lh, in_=lhs_in[v].rearrange("k c r -> c k r"))

                # column sum-of-squares -> (128 cols-in-block, 32 blocks)
                sq0 = sq_pool.tile([128, N], f32)
                nc.vector.tensor_mul(sq0, a0, a0)
                sq1 = sq_pool.tile([128, N], f32)
                nc.scalar.square(sq1, a1)
                ps_ss = pss_pool.tile([128, nblk], f32)
                for j in range(nblk):
                    nc.tensor.matmul(
                        ps_ss[:, j : j + 1],
                        lhsT=sq0[:, j * 128 : (j + 1) * 128],
                        rhs=ones_t,
                        start=True,
                        stop=False,
                    )
                    nc.tensor.matmul(
                        ps_ss[:, j : j + 1],
                        lhsT=sq1[:, j * 128 : (j + 1) * 128],
                        rhs=ones_t,
                        start=False,
                        stop=True,
                    )
                # rnorm10 = 10 / sqrt(ss) = 1 / sqrt(ss * 0.01)
                sq_t = rn_pool.tile([128, nblk], f32)
                nc.scalar.activation(
                    sq_t, ps_ss, mybir.ActivationFunctionType.Sqrt, scale=0.01
                )
                rn10 = rn2_pool.tile([128, nblk], f32)
                nc.vector.reciprocal(rn10, sq_t)

                for g in range(ngrp):
                    ps_s = ps_pool.tile([128, bpg, rows], f32)
                    for jj in range(bpg):
                        j = g * bpg + jj
                        nc.tensor.matmul(
                            ps_s[:, jj, :],
                            lhsT=a0[:, j * 128 : (j + 1) * 128],
                            rhs=lh[:, 0, :],
                            start=True,
                            stop=False,
                        )
                        nc.tensor.matmul(
                            ps_s[:, jj, :],
                            lhsT=a1[:, j * 128 : (j + 1) * 128],
                            rhs=lh[:, 1, :],
                            start=False,
                            stop=True,
                        )
                    sct = sc_pool.tile([128, bpg, rows], f32)
                    for jj in range(bpg):
                        j = g * bpg + jj
                        nc.vector.tensor_scalar(
                            out=sct[:, jj, :],
                            in0=ps_s[:, jj, :],
                            scalar1=rn10[:, j : j + 1],
                            scalar2=None,
                            op0=mybir.AluOpType.mult,
                        )
                    et = e_pool.tile([128, bpg, rows], f32)
                    nc.scalar.activation(et, sct, mybir.ActivationFunctionType.Exp)
                    nc.scalar.dma_start(
                        out=eout[v, g],
                        in_=et.rearrange("p g r -> p (g r)"),
                    )

    nc.compile()
    return nc, bpg, ngrp


def _get_program(rows):
    if rows not in _PROGRAM_CACHE:
        _PROGRAM_CACHE[rows] = _build_program(rows)
    return _PROGRAM_CACHE[rows]


def _install_ntff_hook():
    """Shim antenv.axon_hooks (absent in this container) so that
    run_bass_kernel_spmd(trace=True) can capture NTFF profiles via the
    local libaxon ctypes hook, and keep artifact handling local."""
    import types

    import concourse.bass_utils as bu

    bu.upload_artifacts = lambda tmpdir: tmpdir  # no cloud upload here
    if "antenv.axon_hooks" in sys.modules:
        return
    import antenv

    mod = types.ModuleType("antenv.axon_hooks")
    state = {}
    mod.set_axon_ntff_profile_hook = lambda h: state.__setitem__("h", h)
    mod.get_axon_ntff_profile_hook = lambda: state.get("h")
    sys.modules["antenv.axon_hooks"] = mod
    antenv.axon_hooks = mod
    from trn_agent_boot.trn_boot import _ntff_profile_via_ctypes

    mod.set_axon_ntff_profile_hook(
        _ntff_profile_via_ctypes("/opt/axon/libaxon_pjrt.so")
    )


def _normalize(x):
    n = np.linalg.norm(x, axis=-1, keepdims=True)
    return x / np.maximum(n, 1e-12)


def _pairs(num_targets, k):
    ia, ib = [], []
    shift = 0
    for n in num_targets:
        r = np.arange(int(n) * k)
        aa, bb = np.meshgrid(r, r, indexing="ij")
        ia.append(aa.ravel() + shift)
        ib.append(bb.ravel() + shift)
        shift += int(n) * k
    return np.concatenate(ia), np.concatenate(ib)


def kernel(video_feats, query_feats, sents_feats, iou2d, iou2ds, num_targets, scatter_idx):
    global LAST_EXEC_NS, LAST_RESULTS

    vf4 = np.ascontiguousarray(np.asarray(video_feats, dtype=np.float32))
    qf = np.asarray(query_feats, dtype=np.float32)
    sf = np.asarray(sents_feats, dtype=np.float32)
    iou2d = np.asarray(iou2d, dtype=np.float32)
    iou2ds = np.asarray(iou2ds, dtype=np.float32)
    nt = np.asarray(num_targets).astype(np.int64)
    sc = np.asarray(scatter_idx).astype(np.int64)

    T = iou2ds.shape[0]
    iu0, iu1 = np.triu_indices(D)
    cols = iu0 * D + iu1  # (P,) triu flat column indices

    qfn = _normalize(qf)  # (B, C)
    sfn = _normalize(sf)  # (T, C)
    iou2d_f = iou2d.reshape(B, N)[:, cols]  # (B, P)
    iou2ds_f = iou2ds.reshape(T, N)[:, cols]  # (T, P)

    # top-k (stable ties -> matches jax.lax.top_k)
    topk = np.argsort(-iou2ds_f, axis=1, kind="stable")[:, :K]  # (T, K)
    d0 = iu0[topk]
    d1 = iu1[topk]
    tvraw = vf4[sc[:, None], :, d0, d1]  # (T, K, C)
    tvn = _normalize(tvraw)
    tvf = tvn.reshape(T * K, C)

    # ---- device program ----
    sents_of = [np.where(sc == b)[0] for b in range(B)]
    rmax = max(1, max(len(s) for s in sents_of))
    rows = 32 + rmax * K
    assert rows <= 512, "sentence rows exceed one PSUM bank"

    nc, bpg, ngrp = _get_program(rows)

    lhs_all = np.zeros((NCORES, VPC, 2, 128, rows), np.float32)
    qfT = qfn.T  # (C, B)
    for core in range(NCORES):
        for v in range(VPC):
            b = core * VPC + v
            m = np.zeros((C, rows), np.float32)
            m[:, :B] = qfT
            for j, t in enumerate(sents_of[b]):
                m[:, 32 + j * K : 32 + (j + 1) * K] = tvn[t].T
            lhs_all[core, v, 0] = m[:128]
            lhs_all[core, v, 1] = m[128:]

    in_maps = [
        {
            "vf": vf4[core * VPC : (core + 1) * VPC].reshape(VPC, C, N),
            "lhs": lhs_all[core],
        }
        for core in range(NCORES)
    ]

    trace = bool(int(os.environ.get("TRN_PROFILE", "0")))
    if trace:
        _install_ntff_hook()
        os.makedirs("/tmp/trn_prof", exist_ok=True)
        res = run_bass_kernel_spmd(
            nc, in_maps, list(range(NCORES)), trace=True, tmpdir="/tmp/trn_prof"
        )
    else:
        res = run_bass_kernel_spmd(nc, in_maps, list(range(NCORES)))
    LAST_EXEC_NS = res.exec_time_ns
    LAST_RESULTS = res

    # E[b] : (rows, 4096) exp-scores;  E[b, r, n] = exp(s_r . vhat_b[:,n] / TT)
    E = np.empty((B, rows, N), np.float32)
    for core in range(NCORES):
        eo = np.asarray(res.results[core]["eout"])  # (VPC, ngrp, 128, bpg*rows)
        eo5 = eo.reshape(VPC, ngrp, 128, bpg, rows)
        # column index n = (g*bpg + jj)*128 + p
        E[core * VPC : (core + 1) * VPC] = (
            eo5.transpose(0, 4, 1, 3, 2).reshape(VPC, rows, N)
        )
    Et = E[:, :, cols]  # (B, rows, P) restricted to triu columns

    # ---- host combine (exact reference semantics) ----
    # inter video
    pos = np.einsum("tkc,tc->tk", tvn, qfn[sc])  # (T, K)
    allv = np.einsum("tkc,bc->tkb", tvn, qfn)  # (T, K, B)
    nmask = np.ones((T, B), bool)
    nmask[np.arange(T), sc] = False
    negv = (np.exp(allv / TT) * nmask[:, None, :]).sum(-1)  # (T, K)
    l_iv = np.mean(-(pos / TT - np.log(np.exp(pos / TT) + negv)))

    # inter query: neg over all (video, triu pos) except own-video high-iou
    row_sums = Et[:, :B, :].sum(-1)  # (videos, queries)
    tot = row_sums.sum(0)  # (B,) per query
    own_pos = np.array(
        [Et[b, b, :][iou2d_f[b] > NEG_IOU].sum() for b in range(B)]
    )
    negq = tot - own_pos  # (B,)
    l_iq = np.mean(-(pos / TT - np.log(np.exp(pos / TT) + negq[sc][:, None])))

    # intra video
    rowneg = np.zeros((T, K))
    for b in range(B):
        for j, t in enumerate(sents_of[b]):
            msk = iou2ds_f[t] < NEG_IOU
            rowneg[t, :] = Et[b, 32 + j * K : 32 + (j + 1) * K, :][:, msk].sum(-1)
    rowneg_flat = rowneg.reshape(T * K)
    a_idx, b_idx = _pairs(nt, K)
    pos_pairs = np.sum(tvf[a_idx] * tvf[b_idx], axis=1)
    l_av = np.mean(
        -(pos_pairs / TT - np.log(np.exp(pos_pairs / TT) + rowneg_flat[a_idx]))
    )

    # intra query (tiny, host only)
    multi = nt > 1
    if multi.any():
        sidx = np.nonzero(np.repeat(multi, nt))[0]
        qidx = np.nonzero(multi)[0]
        ms = sfn[sidx]
        msc = sc[sidx]
        iqp = np.sum(ms * qfn[msc], axis=1)
        iqa = ms @ qfn[qidx].T
        pm = np.repeat(np.eye(len(qidx), dtype=bool), nt[multi], axis=0)
        negi = (np.exp(iqa / TT) * ~pm).sum(-1)
        l_aq = np.mean(-(iqp / TT - np.log(np.exp(iqp / TT) + negi)))
    else:
        l_aq = 0.0

    return np.stack([l_iv, l_iq, l_av, l_aq]).astype(np.float32)
